# revision 4
# baseline (speedup 1.0000x reference)
"""Causal self-attention (B=2, T=2048, C=1024, H=16, D=64) on 8 TRN2 cores.

Sharding: core c handles batch b=c//4 and head group g=c%4 (heads 4g..4g+3).
Per core, on device (all matmuls bf16, fp32 PSUM accumulation):
  - qkv projection from pre-transposed x^T (host-prepped, bf16):
      qT/kT in transposed layout [d, m] as head-pairs [128, T];
      V in natural layout [m, d] for the core's 4 heads, with a ones column
      per head appended so the attn@V matmul also produces the softmax
      denominator (row 64 of the [65, m] PSUM output).
  - scores computed transposed S^T[j, m] = kT.T @ qT (1/8 scale folded into
    wq on host); softmax WITHOUT max subtraction (scores are O(5), exact in
    fp32); exp on ScalarE straight out of PSUM; causal handled by skipping
    fully-masked blocks and one affine_select on each diagonal block.
  - y^T normalized via a K=1 broadcast matmul of 1/l, stays transposed as
    the c_proj stationary; c_proj partial output [T, C] in fp32.
Host: sums the 4 partials per batch, adds b_proj and the v-bias term.
"""

import math
import numpy as np
import ml_dtypes

import concourse.bass as bass
import concourse.tile as tile
from concourse import bacc, mybir
from concourse.bass_utils import run_bass_kernel_spmd

BF16 = ml_dtypes.bfloat16
F32 = mybir.dt.float32
BF = mybir.dt.bfloat16

B, T, C = 2, 2048, 1024
H, D = 16, 64
N_CORES = 8
GROUPS = 4            # head groups (cores per batch)
HPC = 4               # heads per core
CC = 8                # contraction chunks: C / 128
MB = T // 128         # 16 m-blocks
MC = T // 512         # 4 m-chunks


DEFAULT_OPTS = dict(
    stages=("qkv", "attn", "cproj"),
    qkv_bufs=3, s_bufs=3, y_bufs=2, bc_bufs=1, o_bufs=1,
    expS_bufs=8, out_bufs=3, const_bufs=1, work_bufs=1,
    bcast_engine="vector", out_copy_engine="vector",
)


def emit_body(nc, tc, ctx_pools, xt_ap, wqk_ap, wv_ap, wp_ap, bqk_ap, outp_ap,
              opts=DEFAULT_OPTS):
    (const_pool, qkT_pool, v_pool, yT_pool, expS_pool, out_pool,
     recip_pool, bcast_pool) = ctx_pools

    # per-chunk tiles so compute can start as soon as each chunk's DMA lands
    # spread loads over SP-HWDGE, ACT-HWDGE and Pool-SWDGE queues
    xt, wqk, wv = [], [], []
    for cc in range(CC):
        xtc = const_pool.tile([128, T], BF, tag=f"xt{cc}")
        (nc.sync if cc % 2 == 0 else nc.scalar).dma_start(
            xtc[:], xt_ap[cc * 128:(cc + 1) * 128, :])
        xt.append(xtc)
        wqc = const_pool.tile([128, 512], BF, tag=f"wqk{cc}")
        (nc.sync if cc % 2 == 1 else nc.scalar).dma_start(
            wqc[:], wqk_ap[cc * 128:(cc + 1) * 128, :])
        wqk.append(wqc)
        wvc = const_pool.tile([128, 256], BF, tag=f"wv{cc}")
        (nc.sync if cc % 2 == 1 else nc.scalar).dma_start(
            wvc[:], wv_ap[cc * 128:(cc + 1) * 128, :])
        wv.append(wvc)
    wp = const_pool.tile([128, 2 * 1024], BF, tag="wp")
    for p2 in range(2):
        nc.sync.dma_start(wp[:, p2 * 1024:(p2 + 1) * 1024], wp_ap[p2 * 128:(p2 + 1) * 128, :])
    bqk = const_pool.tile([128, 4], F32, tag="bqk")
    nc.sync.dma_start(bqk[:], bqk_ap[:])
    ones = const_pool.tile([1, 64], BF, tag="ones")
    nc.gpsimd.memset(ones[:], 1.0)
    # lower-triangular (keep j<=m) bf16 mask for diagonal score blocks
    cmask = const_pool.tile([128, 128], BF, tag="cmask")
    nc.gpsimd.memset(cmask[:], 1.0)
    nc.gpsimd.affine_select(
        out=cmask[:], in_=cmask[:], compare_op=mybir.AluOpType.is_ge,
        fill=0.0, base=0, pattern=[[1, 128]], channel_multiplier=-1)

    qkT = qkT_pool.tile([128, 4 * T], BF, tag="qkT")   # q pair0, q pair1, k pair0, k pair1
    v_all = v_pool.tile([128, MB * 260], BF, tag="v")  # per m-block: 4x(64 v cols + ones col)
    yT = yT_pool.tile([128, 2 * T], BF, tag="yT")      # per pair: [hd, m]

    # ---- qkv projection ----
    if "qkv" not in opts["stages"]:
        return
    with tc.tile_pool(name="qkv_ps", bufs=opts["qkv_bufs"], space="PSUM") as qkv_psum:
        for pp in range(4):          # q0 q1 k0 k1 pair outputs
            for mc in range(MC):
                ps = qkv_psum.tile([128, 512], F32, tag="qkvps")
                for cc in range(CC):
                    nc.tensor.matmul(
                        ps[:],
                        lhsT=wqk[cc][:, pp * 128:(pp + 1) * 128],
                        rhs=xt[cc][:, mc * 512:(mc + 1) * 512],
                        start=(cc == 0), stop=(cc == CC - 1))
                nc.vector.tensor_scalar_add(
                    qkT[:, pp * T + mc * 512: pp * T + (mc + 1) * 512],
                    ps[:], bqk[:, pp:pp + 1])
        for mb in range(MB):
            ps = qkv_psum.tile([128, 512], F32, tag="qkvps")
            for cc in range(CC):
                nc.tensor.matmul(
                    ps[:, 0:256],
                    lhsT=xt[cc][:, mb * 128:(mb + 1) * 128],
                    rhs=wv[cc][:],
                    start=(cc == 0), stop=(cc == CC - 1))
            vb = v_all[:, mb * 260:(mb + 1) * 260]
            nc.gpsimd.memset(vb.rearrange("p (h x) -> p h x", x=65)[:, :, 64:65], 1.0)
            nc.vector.tensor_copy(
                vb.rearrange("p (h x) -> p h x", x=65)[:, :, 0:64],
                ps[:, 0:256].rearrange("p (h x) -> p h x", x=64))

    # ---- attention (m-chunk outer, head inner) with c_proj inlined ----
    if "attn" not in opts["stages"]:
        return
    do_cproj = "cproj" in opts["stages"]
    with (
        tc.tile_pool(name="s_ps", bufs=opts["s_bufs"], space="PSUM") as s_psum,
        tc.tile_pool(name="y_ps", bufs=opts["y_bufs"], space="PSUM") as y_psum,
        tc.tile_pool(name="bc_ps", bufs=opts["bc_bufs"], space="PSUM") as bc_psum,
        tc.tile_pool(name="o_ps", bufs=opts["o_bufs"], space="PSUM") as o_psum,
    ):
        for mc in range(MC):
            for h in range(HPC):
                pp, half = h // 2, h % 2
                prow = 64 * half
                qoff = pp * T
                koff = (2 + pp) * T
                yp = y_psum.tile([65, 512], F32, tag="ypsum")
                last_jb = 4 * mc + 3
                for jb in range(4 * mc + 4):
                    off = max(0, (jb - 4 * mc) * 128)
                    w = 512 - off
                    m_abs = mc * 512 + off
                    sp = s_psum.tile([128, 512], F32, tag="spsum")
                    nc.tensor.matmul(
                        sp[:, 0:w],
                        lhsT=qkT[prow:prow + 64, koff + jb * 128: koff + (jb + 1) * 128],
                        rhs=qkT[prow:prow + 64, qoff + m_abs: qoff + m_abs + w],
                        start=True, stop=True)
                    es = expS_pool.tile([128, 512], BF, tag="expS")
                    nc.scalar.activation(es[:, 0:w], sp[:, 0:w],
                                         mybir.ActivationFunctionType.Exp)
                    if jb >= 4 * mc:
                        nc.vector.tensor_mul(es[:, 0:128], es[:, 0:128], cmask[:])
                    nc.tensor.matmul(
                        yp[:, off:512],
                        lhsT=v_all[:, jb * 260 + h * 65: jb * 260 + (h + 1) * 65],
                        rhs=es[:, 0:w],
                        start=(jb == 0), stop=(jb == last_jb))
                rc = recip_pool.tile([1, 512], BF, tag="recip")
                with nc.allow_low_precision(reason="1/l broadcastee; bf16 ok"):
                    nc.vector.reciprocal(rc[:], yp[64:65, :])
                bc = bc_psum.tile([64, 512], F32, tag="bcps")
                nc.tensor.matmul(bc[:], lhsT=ones[:], rhs=rc[:], start=True, stop=True)
                bs = bcast_pool.tile([64, 512], F32, tag="bcsb")
                if opts["bcast_engine"] == "scalar":
                    nc.scalar.activation(bs[:], bc[:], mybir.ActivationFunctionType.Copy)
                else:
                    nc.vector.tensor_copy(bs[:], bc[:])
                nc.vector.tensor_mul(
                    yT[prow:prow + 64, pp * T + mc * 512: pp * T + (mc + 1) * 512],
                    yp[0:64, :], bs[:])
            if not do_cproj:
                continue
            for mb in range(4 * mc, 4 * mc + 4):
                op = o_psum.tile([128, 1024], F32, tag="opsum")
                for pp2 in range(2):
                    for nch in range(2):
                        nc.tensor.matmul(
                            op[:, nch * 512:(nch + 1) * 512],
                            lhsT=yT[:, pp2 * T + mb * 128: pp2 * T + (mb + 1) * 128],
                            rhs=wp[:, pp2 * 1024 + nch * 512: pp2 * 1024 + (nch + 1) * 512],
                            start=(pp2 == 0), stop=(pp2 == 1))
                ob = out_pool.tile([128, 1024], BF, tag="outsb")
                if opts["out_copy_engine"] == "vector":
                    nc.vector.tensor_copy(ob[:], op[:])
                else:
                    nc.scalar.activation(ob[:], op[:], mybir.ActivationFunctionType.Copy)
                nc.sync.dma_start(outp_ap[mb * 128:(mb + 1) * 128, :], ob[:])


def build(reps=1, opts=None):
    opts = {**DEFAULT_OPTS, **(opts or {})}
    nc = bacc.Bacc("TRN2", target_bir_lowering=False, debug=False)
    xt_ap = nc.dram_tensor("xt", [C, T], BF, kind="ExternalInput").ap()
    wqk_ap = nc.dram_tensor("wqk", [C, 512], BF, kind="ExternalInput").ap()
    wv_ap = nc.dram_tensor("wv", [C, 256], BF, kind="ExternalInput").ap()
    wp_ap = nc.dram_tensor("wp", [256, 1024], BF, kind="ExternalInput").ap()
    bqk_ap = nc.dram_tensor("bqk", [128, 4], F32, kind="ExternalInput").ap()
    outp_ap = nc.dram_tensor("outp", [T, C], BF, kind="ExternalOutput").ap()

    with tile.TileContext(nc) as tc:
        with (
            tc.tile_pool(name="const", bufs=opts["const_bufs"]) as const_pool,
            tc.tile_pool(name="qkT", bufs=opts["work_bufs"]) as qkT_pool,
            tc.tile_pool(name="v", bufs=opts["work_bufs"]) as v_pool,
            tc.tile_pool(name="yT", bufs=opts["work_bufs"]) as yT_pool,
            tc.tile_pool(name="expS", bufs=opts["expS_bufs"]) as expS_pool,
            tc.tile_pool(name="outsb", bufs=opts["out_bufs"]) as out_pool,
            tc.tile_pool(name="recip", bufs=2) as recip_pool,
            tc.tile_pool(name="bcast", bufs=2) as bcast_pool,
        ):
            pools = (const_pool, qkT_pool, v_pool, yT_pool, expS_pool,
                     out_pool, recip_pool, bcast_pool)
            for _ in range(reps):
                emit_body(nc, tc, pools, xt_ap, wqk_ap, wv_ap, wp_ap, bqk_ap, outp_ap, opts)
    nc.compile()
    return nc


def build_looped(n_iters, opts=None):
    """Body wrapped in a hardware For_i loop, for wall-clock slope timing."""
    opts = {**DEFAULT_OPTS, **(opts or {})}
    nc = bacc.Bacc("TRN2", target_bir_lowering=False, debug=False)
    xt_ap = nc.dram_tensor("xt", [C, T], BF, kind="ExternalInput").ap()
    wqk_ap = nc.dram_tensor("wqk", [C, 512], BF, kind="ExternalInput").ap()
    wv_ap = nc.dram_tensor("wv", [C, 256], BF, kind="ExternalInput").ap()
    wp_ap = nc.dram_tensor("wp", [256, 1024], BF, kind="ExternalInput").ap()
    bqk_ap = nc.dram_tensor("bqk", [128, 4], F32, kind="ExternalInput").ap()
    outp_ap = nc.dram_tensor("outp", [T, C], BF, kind="ExternalOutput").ap()
    with tile.TileContext(nc) as tc:
        with (
            tc.tile_pool(name="const", bufs=opts["const_bufs"]) as const_pool,
            tc.tile_pool(name="qkT", bufs=opts["work_bufs"]) as qkT_pool,
            tc.tile_pool(name="v", bufs=opts["work_bufs"]) as v_pool,
            tc.tile_pool(name="yT", bufs=opts["work_bufs"]) as yT_pool,
            tc.tile_pool(name="expS", bufs=opts["expS_bufs"]) as expS_pool,
            tc.tile_pool(name="outsb", bufs=opts["out_bufs"]) as out_pool,
            tc.tile_pool(name="recip", bufs=2) as recip_pool,
            tc.tile_pool(name="bcast", bufs=2) as bcast_pool,
        ):
            pools = (const_pool, qkT_pool, v_pool, yT_pool, expS_pool,
                     out_pool, recip_pool, bcast_pool)
            with tc.For_i(0, n_iters, 1):
                emit_body(nc, tc, pools, xt_ap, wqk_ap, wv_ap, wp_ap, bqk_ap,
                          outp_ap, opts)
    nc.compile()
    return nc


_NC_CACHE = {}


def _get_nc(reps=1, opts=None):
    key = (reps, tuple(sorted((opts or {}).items())))
    if key not in _NC_CACHE:
        _NC_CACHE[key] = build(reps, opts)
    return _NC_CACHE[key]


def make_in_maps(x, w_attn, b_attn, w_proj):
    x = np.asarray(x, np.float32)
    w_attn = np.asarray(w_attn, np.float32)
    b_attn = np.asarray(b_attn, np.float32)
    in_maps = []
    xt_b = [np.ascontiguousarray(x[b].T).astype(BF16) for b in range(B)]
    for c in range(N_CORES):
        b, g = divmod(c, GROUPS)
        h0 = HPC * g
        qs, ks = h0 * D, C + h0 * D
        wqk = np.concatenate([
            0.125 * w_attn[:, qs:qs + 128], 0.125 * w_attn[:, qs + 128:qs + 256],
            w_attn[:, ks:ks + 128], w_attn[:, ks + 128:ks + 256]], axis=1).astype(BF16)
        wv = w_attn[:, 2 * C + g * 256: 2 * C + (g + 1) * 256].astype(BF16)
        wp = np.asarray(w_proj, np.float32)[g * 256:(g + 1) * 256, :].astype(BF16)
        bqk = np.stack([
            0.125 * b_attn[qs:qs + 128], 0.125 * b_attn[qs + 128:qs + 256],
            b_attn[ks:ks + 128], b_attn[ks + 128:ks + 256]], axis=1).astype(np.float32)
        in_maps.append({"xt": xt_b[b], "wqk": np.ascontiguousarray(wqk),
                        "wv": np.ascontiguousarray(wv), "wp": np.ascontiguousarray(wp),
                        "bqk": np.ascontiguousarray(bqk)})
    return in_maps


def assemble_output(results, b_attn, w_proj, b_proj):
    b_attn = np.asarray(b_attn, np.float32)
    w_proj = np.asarray(w_proj, np.float32)
    b_proj = np.asarray(b_proj, np.float32)
    extra = b_attn[2 * C:] @ w_proj + b_proj  # v-bias flows through softmax as +bv
    out = np.empty((B, T, C), np.float32)
    for b in range(B):
        acc = results[4 * b]["outp"].astype(np.float32).copy()
        for g in range(1, GROUPS):
            acc += results[4 * b + g]["outp"]
        out[b] = acc + extra
    return out


def kernel(x, w_attn, b_attn, w_proj, b_proj):
    nc = _get_nc(reps=1)
    in_maps = make_in_maps(x, w_attn, b_attn, w_proj)
    res = run_bass_kernel_spmd(nc, in_maps, list(range(N_CORES)))
    return assemble_output(res.results, b_attn, w_proj, b_proj)



# revision 5
# speedup vs baseline: 1.5958x; 1.5958x over previous
"""Causal self-attention (B=2, T=2048, C=1024, H=16, D=64) on 8 TRN2 cores. v2.

Sharding: core c handles batch b=c//4 and head group g=c%4 (heads 4g..4g+3).

Changes vs the original baseline kernel:
  - Single PSUM pool scope for the whole kernel; qkv and attention emitted
    interleaved per m-chunk (qkv of chunk mc+1 and c_proj of chunk mc-1 are
    spread as PE filler between attention score/V pairs) so exp work on the
    Activation engine overlaps PE work across the full kernel.
  - Score j-blocks processed in PAIRS sharing one [128,1024] PSUM tile; one
    exp instruction per pair (halves Activation instruction count: 160->80).
  - qkv pair-groups share [128,1024] PSUM tiles; V for 4 m-blocks computed
    into one tile and interleaved into v_all with a single vector copy.
  - c_proj partial outputs written as bf16 (halves output DMA; host sums in
    fp32), emitted as independent 512-wide pieces.
  - Input DMAs, causal masking (cmask multiply on DVE) and 1/l broadcast
    (K=1 ones matmul + Act copy) use the baseline's proven HW paths.
"""

import math
import numpy as np
import ml_dtypes

import concourse.bass as bass
import concourse.tile as tile
from concourse import bacc, mybir
from concourse.bass_utils import run_bass_kernel_spmd

BF16 = ml_dtypes.bfloat16
F32 = mybir.dt.float32
BF = mybir.dt.bfloat16

B, T, C = 2, 2048, 1024
H, D = 16, 64
N_CORES = 8
GROUPS = 4            # head groups (cores per batch)
HPC = 4               # heads per core
CC = 8                # contraction chunks: C / 128
MB = T // 128         # 16 m-blocks
MC = T // 512         # 4 m-chunks


DEFAULT_OPTS = dict(
    stages=("qkv", "attn", "cproj"),
    sq_bufs=2, y_bufs=2, o_bufs=1,
    es_bufs=8, out_bufs=3,
)


EMIT_LOG = []  # (label, first_inst_id, last_inst_id) for profiling


def emit_body(nc, tc, pools, xt_ap, wqk_ap, wv_ap, wp_ap, bqk_ap, outp_ap,
              opts=DEFAULT_OPTS):
    (const_pool, qkT_pool, v_pool, yT_pool, es_pool, out_pool,
     rc_pool, bs_pool, sq_psum, y_psum, o_psum, bc_psum) = pools
    do_qkv = "qkv" in opts["stages"]
    do_attn = "attn" in opts["stages"]
    do_cproj = "cproj" in opts["stages"]

    # ---- constant loads (baseline recipe: full-width chunks on the SP- and
    # Act-HWDGE queues; per-chunk tiles so compute starts as chunks land) ----
    xt, wqk, wv = [], [], []
    for cc in range(CC):
        xtc = const_pool.tile([128, T], BF, tag=f"xt{cc}")
        (nc.sync if cc % 2 == 0 else nc.scalar).dma_start(
            xtc[:], xt_ap[cc * 128:(cc + 1) * 128, :])
        xt.append(xtc)
        wqc = const_pool.tile([128, 512], BF, tag=f"wqk{cc}")
        (nc.sync if cc % 2 == 1 else nc.scalar).dma_start(
            wqc[:], wqk_ap[cc * 128:(cc + 1) * 128, :])
        wqk.append(wqc)
        wvc = const_pool.tile([128, 256], BF, tag=f"wv{cc}")
        (nc.sync if cc % 2 == 1 else nc.scalar).dma_start(
            wvc[:], wv_ap[cc * 128:(cc + 1) * 128, :])
        wv.append(wvc)
    wp = const_pool.tile([128, 2 * 1024], BF, tag="wp")
    for p2 in range(2):
        nc.sync.dma_start(wp[:, p2 * 1024:(p2 + 1) * 1024],
                          wp_ap[p2 * 128:(p2 + 1) * 128, :])
    bqk = const_pool.tile([128, 4], F32, tag="bqk")
    nc.sync.dma_start(bqk[:], bqk_ap[:])

    qkT = qkT_pool.tile([128, 4 * T], BF, tag="qkT")   # q pair0, q pair1, k pair0, k pair1
    v_all = v_pool.tile([128, MB * 260], BF, tag="v")  # per m-block: 4x(64 v cols + ones col)
    yT = yT_pool.tile([128, 2 * T], BF, tag="yT")      # per pair: [hd, m]
    ones = const_pool.tile([1, 64], BF, tag="ones")
    nc.gpsimd.memset(ones[:], 1.0)
    # lower-triangular (keep j<=m) bf16 mask for diagonal score blocks
    cmask = const_pool.tile([128, 128], BF, tag="cmask")
    nc.gpsimd.memset(cmask[:], 1.0)
    nc.gpsimd.affine_select(
        out=cmask[:], in_=cmask[:], compare_op=mybir.AluOpType.is_ge,
        fill=0.0, base=0, pattern=[[1, 128]], channel_multiplier=-1)

    # ones columns of v_all (one memset; disjoint from the v columns)
    nc.gpsimd.memset(
        v_all.rearrange("p (mb h x) -> p mb h x", h=4, x=65)[:, :, :, 64:65], 1.0)

    def _qkv_pg_half(ps, mc, pg, half):
        pp = 2 * pg + half
        for cc in range(CC):
            nc.tensor.matmul(
                ps[:, half * 512:(half + 1) * 512],
                lhsT=wqk[cc][:, pp * 128:(pp + 1) * 128],
                rhs=xt[cc][:, mc * 512:(mc + 1) * 512],
                start=(cc == 0), stop=(cc == CC - 1))

    def qkv_pg_pieces(mc, pg):
        # pair-group pg: pp = 2*pg, 2*pg+1 (q pairs for pg=0, k pairs for pg=1)
        st = {}

        def piece0():
            ps_t = sq_psum.tile([128, 1024], F32, tag="sq")
            st["ps"] = ps_t
            _qkv_pg_half(ps_t, mc, pg, 0)

        def piece1():
            ps = st["ps"]
            _qkv_pg_half(ps, mc, pg, 1)
            for half in range(2):
                pp = 2 * pg + half
                nc.vector.tensor_scalar_add(
                    qkT[:, pp * T + mc * 512: pp * T + (mc + 1) * 512],
                    ps[:, half * 512:(half + 1) * 512], bqk[:, pp:pp + 1])

        return [(f"qkv{mc}:pg{pg}a", piece0), (f"qkv{mc}:pg{pg}b", piece1)]

    def _qkv_v_half(ps, mc, qlo):
        for q in (qlo, qlo + 1):
            mb = 4 * mc + q
            for cc in range(CC):
                nc.tensor.matmul(
                    ps[:, q * 256:(q + 1) * 256],
                    lhsT=xt[cc][:, mb * 128:(mb + 1) * 128],
                    rhs=wv[cc][:],
                    start=(cc == 0), stop=(cc == CC - 1))

    def qkv_v_pieces(mc):
        st = {}

        def piece0():
            ps_t = sq_psum.tile([128, 1024], F32, tag="sq")
            st["ps"] = ps_t
            _qkv_v_half(ps_t, mc, 0)

        def piece1():
            ps = st["ps"]
            _qkv_v_half(ps, mc, 2)
            nc.vector.tensor_copy(
                v_all.rearrange("p (mb h x) -> p mb h x", h=4, x=65)[
                    :, 4 * mc:4 * mc + 4, :, 0:64],
                ps[:].rearrange("p (q h x) -> p q h x", h=4, x=64))

        return [(f"qkv{mc}:va", piece0), (f"qkv{mc}:vb", piece1)]

    def emit_qkv_all(mc):
        for lb, fn in qkv_pg_pieces(mc, 0) + qkv_pg_pieces(mc, 1) \
                + qkv_v_pieces(mc):
            fn()

    def _cproj_piece(mb, nch):
        op = o_psum.tile([128, 512], F32, tag="opsum")
        for pp2 in range(2):
            nc.tensor.matmul(
                op[:],
                lhsT=yT[:, pp2 * T + mb * 128: pp2 * T + (mb + 1) * 128],
                rhs=wp[:, pp2 * 1024 + nch * 512: pp2 * 1024 + (nch + 1) * 512],
                start=(pp2 == 0), stop=(pp2 == 1))
        ob = out_pool.tile([128, 512], BF, tag="outsb")
        nc.vector.tensor_copy(ob[:], op[:])
        nc.sync.dma_start(
            outp_ap[mb * 128:(mb + 1) * 128, nch * 512:(nch + 1) * 512],
            ob[:])

    def cproj_pieces(mb):
        return [(f"cproj:mb{mb}n{nch}",
                 lambda nch=nch: _cproj_piece(mb, nch)) for nch in range(2)]

    def emit_attn_pair(mc, h, p):
        pp, half = h // 2, h % 2
        prow = 64 * half
        qoff = pp * T
        koff = (2 + pp) * T
        yp = yps[h]
        n_pairs = 2 * mc + 2
        jb0 = 2 * p
        if p < 2 * mc:
            off0 = off1 = 0
            w0 = w1 = 512
        else:
            off0 = (jb0 - 4 * mc) * 128
            w0 = 512 - off0
            w1 = w0 - 128
            off1 = off0 + 128
        sp = sq_psum.tile([128, 1024], F32, tag="sq")
        nc.tensor.matmul(
            sp[:, 0:w0],
            lhsT=qkT[prow:prow + 64,
                     koff + jb0 * 128: koff + (jb0 + 1) * 128],
            rhs=qkT[prow:prow + 64,
                    qoff + mc * 512 + off0: qoff + mc * 512 + off0 + w0],
            start=True, stop=True)
        nc.tensor.matmul(
            sp[:, w0:w0 + w1],
            lhsT=qkT[prow:prow + 64,
                     koff + (jb0 + 1) * 128: koff + (jb0 + 2) * 128],
            rhs=qkT[prow:prow + 64,
                    qoff + mc * 512 + off1: qoff + mc * 512 + off1 + w1],
            start=True, stop=True)
        es = es_pool.tile([128, 1024], BF, tag="es")
        nc.scalar.activation(es[:, 0:w0 + w1], sp[:, 0:w0 + w1],
                             mybir.ActivationFunctionType.Exp)
        if p >= 2 * mc:
            # zero the upper-triangular parts of both 128-col diagonal
            # triangles (jb0 at col 0, jb1 at col w0)
            nc.vector.tensor_mul(es[:, 0:128], es[:, 0:128], cmask[:])
            nc.vector.tensor_mul(es[:, w0:w0 + 128], es[:, w0:w0 + 128],
                                 cmask[:])
        nc.tensor.matmul(
            yp[:, off0:512],
            lhsT=v_all[:, jb0 * 260 + h * 65: jb0 * 260 + (h + 1) * 65],
            rhs=es[:, 0:w0],
            start=(p == 0), stop=False)
        nc.tensor.matmul(
            yp[:, off1:512],
            lhsT=v_all[:, (jb0 + 1) * 260 + h * 65:
                       (jb0 + 1) * 260 + (h + 1) * 65],
            rhs=es[:, w0:w0 + w1],
            start=False, stop=(p == n_pairs - 1))

    def emit_normalize(mc, h):
        pp, half = h // 2, h % 2
        prow = 64 * half
        yp = yps[h]
        rc = rc_pool.tile([1, 512], BF, tag="recip")
        with nc.allow_low_precision(reason="1/l broadcastee; bf16 ok"):
            nc.vector.reciprocal(rc[:], yp[64:65, :])
        bc = bc_psum.tile([64, 512], F32, tag="bcps")
        nc.tensor.matmul(bc[:], lhsT=ones[:], rhs=rc[:], start=True, stop=True)
        bs = bs_pool.tile([64, 512], F32, tag="bs")
        nc.scalar.activation(bs[:], bc[:], mybir.ActivationFunctionType.Copy)
        nc.vector.tensor_mul(
            yT[prow:prow + 64, pp * T + mc * 512: pp * T + (mc + 1) * 512],
            yp[0:64, :], bs[:])

    if not do_attn:
        if do_qkv:
            for mc in range(MC):
                emit_qkv_all(mc)
        return

    def logged(label, fn, *args):
        i0 = nc.next_id()
        fn(*args)
        EMIT_LOG.append((label, i0, nc.next_id()))

    # qkv(0) up front; then per mc: attention pairs with qkv(mc+1) and
    # cproj(mc-1) interleaved as PE filler between pairs.
    logged("qkv0", emit_qkv_all, 0)
    yps = {}
    for mc in range(MC):
        filler = []
        if do_qkv and mc + 1 < MC:
            filler += qkv_pg_pieces(mc + 1, 0) + qkv_pg_pieces(mc + 1, 1) \
                + qkv_v_pieces(mc + 1)
        if do_cproj and mc > 0:
            for mb in range(4 * (mc - 1), 4 * mc):
                filler += cproj_pieces(mb)
        n_pairs = 2 * mc + 2
        total_pairs = HPC * n_pairs
        stride = max(1, total_pairs // (len(filler) + 1))
        cnt = 0
        for h in range(HPC):
            yp_t = y_psum.tile([65, 512], F32, tag="ypsum")
            yps[h] = yp_t
            for p in range(n_pairs):
                logged(f"attn{mc}:h{h}p{p}", emit_attn_pair, mc, h, p)
                cnt += 1
                if cnt % stride == 0 and filler:
                    lb, fn = filler.pop(0)
                    logged(lb, fn)
            logged(f"norm{mc}:h{h}", emit_normalize, mc, h)
        while filler:
            lb, fn = filler.pop(0)
            logged(lb, fn)
    if do_cproj:
        for mb in range(4 * (MC - 1), 4 * MC):
            for lb, fn in cproj_pieces(mb):
                logged(lb, fn)


def _build_pools(nc, tc, ctx, opts):
    const_pool = ctx.enter_context(tc.tile_pool(name="const", bufs=1))
    qkT_pool = ctx.enter_context(tc.tile_pool(name="qkT", bufs=1))
    v_pool = ctx.enter_context(tc.tile_pool(name="v", bufs=1))
    yT_pool = ctx.enter_context(tc.tile_pool(name="yT", bufs=1))
    es_pool = ctx.enter_context(tc.tile_pool(name="es", bufs=opts["es_bufs"]))
    out_pool = ctx.enter_context(tc.tile_pool(name="outsb", bufs=opts["out_bufs"]))
    rc_pool = ctx.enter_context(tc.tile_pool(name="recip", bufs=2))
    bs_pool = ctx.enter_context(tc.tile_pool(name="bs", bufs=2))
    sq_psum = ctx.enter_context(
        tc.tile_pool(name="sq_ps", bufs=opts["sq_bufs"], space="PSUM"))
    y_psum = ctx.enter_context(
        tc.tile_pool(name="y_ps", bufs=opts["y_bufs"], space="PSUM"))
    o_psum = ctx.enter_context(
        tc.tile_pool(name="o_ps", bufs=opts["o_bufs"], space="PSUM"))
    bc_psum = ctx.enter_context(
        tc.tile_pool(name="bc_ps", bufs=1, space="PSUM"))
    return (const_pool, qkT_pool, v_pool, yT_pool, es_pool, out_pool,
            rc_pool, bs_pool, sq_psum, y_psum, o_psum, bc_psum)


def _declare_io(nc):
    xt_ap = nc.dram_tensor("xt", [C, T], BF, kind="ExternalInput").ap()
    wqk_ap = nc.dram_tensor("wqk", [C, 512], BF, kind="ExternalInput").ap()
    wv_ap = nc.dram_tensor("wv", [C, 256], BF, kind="ExternalInput").ap()
    wp_ap = nc.dram_tensor("wp", [256, 1024], BF, kind="ExternalInput").ap()
    bqk_ap = nc.dram_tensor("bqk", [128, 4], F32, kind="ExternalInput").ap()
    outp_ap = nc.dram_tensor("outp", [T, C], BF, kind="ExternalOutput").ap()
    return xt_ap, wqk_ap, wv_ap, wp_ap, bqk_ap, outp_ap


def build(reps=1, opts=None):
    from contextlib import ExitStack
    opts = {**DEFAULT_OPTS, **(opts or {})}
    nc = bacc.Bacc("TRN2", target_bir_lowering=False, debug=False)
    aps = _declare_io(nc)
    with tile.TileContext(nc) as tc:
        with ExitStack() as ctx:
            pools = _build_pools(nc, tc, ctx, opts)
            for _ in range(reps):
                emit_body(nc, tc, pools, *aps, opts)
    nc.compile()
    return nc


def build_looped(n_iters, opts=None):
    """Body wrapped in a hardware For_i loop, for wall-clock slope timing."""
    from contextlib import ExitStack
    opts = {**DEFAULT_OPTS, **(opts or {})}
    nc = bacc.Bacc("TRN2", target_bir_lowering=False, debug=False)
    aps = _declare_io(nc)
    with tile.TileContext(nc) as tc:
        with ExitStack() as ctx:
            pools = _build_pools(nc, tc, ctx, opts)
            with tc.For_i(0, n_iters, 1):
                emit_body(nc, tc, pools, *aps, opts)
    nc.compile()
    return nc


_NC_CACHE = {}


def _get_nc(reps=1, opts=None):
    key = (reps, tuple(sorted((opts or {}).items())))
    if key not in _NC_CACHE:
        _NC_CACHE[key] = build(reps, opts)
    return _NC_CACHE[key]


def make_in_maps(x, w_attn, b_attn, w_proj):
    x = np.asarray(x, np.float32)
    w_attn = np.asarray(w_attn, np.float32)
    b_attn = np.asarray(b_attn, np.float32)
    in_maps = []
    xt_b = [np.ascontiguousarray(x[b].T).astype(BF16) for b in range(B)]
    for c in range(N_CORES):
        b, g = divmod(c, GROUPS)
        h0 = HPC * g
        qs, ks = h0 * D, C + h0 * D
        wqk = np.concatenate([
            0.125 * w_attn[:, qs:qs + 128], 0.125 * w_attn[:, qs + 128:qs + 256],
            w_attn[:, ks:ks + 128], w_attn[:, ks + 128:ks + 256]], axis=1).astype(BF16)
        wv = w_attn[:, 2 * C + g * 256: 2 * C + (g + 1) * 256].astype(BF16)
        wp = np.asarray(w_proj, np.float32)[g * 256:(g + 1) * 256, :].astype(BF16)
        bqk = np.stack([
            0.125 * b_attn[qs:qs + 128], 0.125 * b_attn[qs + 128:qs + 256],
            b_attn[ks:ks + 128], b_attn[ks + 128:ks + 256]], axis=1).astype(np.float32)
        in_maps.append({"xt": xt_b[b], "wqk": np.ascontiguousarray(wqk),
                        "wv": np.ascontiguousarray(wv), "wp": np.ascontiguousarray(wp),
                        "bqk": np.ascontiguousarray(bqk)})
    return in_maps


def assemble_output(results, b_attn, w_proj, b_proj):
    b_attn = np.asarray(b_attn, np.float32)
    w_proj = np.asarray(w_proj, np.float32)
    b_proj = np.asarray(b_proj, np.float32)
    extra = b_attn[2 * C:] @ w_proj + b_proj  # v-bias flows through softmax as +bv
    out = np.empty((B, T, C), np.float32)
    for b in range(B):
        acc = results[4 * b]["outp"].astype(np.float32)
        for g in range(1, GROUPS):
            acc = acc + results[4 * b + g]["outp"].astype(np.float32)
        out[b] = acc + extra
    return out


def kernel(x, w_attn, b_attn, w_proj, b_proj):
    nc = _get_nc(reps=1)
    in_maps = make_in_maps(x, w_attn, b_attn, w_proj)
    res = run_bass_kernel_spmd(nc, in_maps, list(range(N_CORES)))
    return assemble_output(res.results, b_attn, w_proj, b_proj)


# revision 6
# speedup vs baseline: 1.7050x; 1.0684x over previous
"""Causal self-attention (B=2, T=2048, C=1024, H=16, D=64) on 8 TRN2 cores. v2.

Sharding: core c handles batch b=c//4 and head group g=c%4 (heads 4g..4g+3).

Changes vs the original baseline kernel:
  - Single PSUM pool scope for the whole kernel; qkv and attention emitted
    interleaved per m-chunk (qkv of chunk mc+1 and c_proj of chunk mc-1 are
    spread as PE filler between attention score/V pairs) so exp work on the
    Activation engine overlaps PE work across the full kernel.
  - Score j-blocks processed in PAIRS sharing one [128,1024] PSUM tile; one
    exp instruction per pair (halves Activation instruction count: 160->80).
  - qkv pair-groups share [128,1024] PSUM tiles; V for 4 m-blocks computed
    into one tile and interleaved into v_all with a single vector copy.
  - c_proj partial outputs written as bf16 (halves output DMA; host sums in
    fp32), emitted as independent 512-wide pieces.
  - Input DMAs, causal masking (cmask multiply on DVE) and 1/l broadcast
    (K=1 ones matmul + Act copy) use the baseline's proven HW paths.
"""

import math
import numpy as np
import ml_dtypes

import concourse.bass as bass
import concourse.tile as tile
from concourse import bacc, mybir
from concourse.bass_utils import run_bass_kernel_spmd

BF16 = ml_dtypes.bfloat16
F32 = mybir.dt.float32
BF = mybir.dt.bfloat16

B, T, C = 2, 2048, 1024
H, D = 16, 64
N_CORES = 8
GROUPS = 4            # head groups (cores per batch)
HPC = 4               # heads per core
CC = 8                # contraction chunks: C / 128
MB = T // 128         # 16 m-blocks
MC = T // 512         # 4 m-chunks


DEFAULT_OPTS = dict(
    stages=("qkv", "attn", "cproj"),
    sq_bufs=2, y_bufs=2, o_bufs=1,
    es_bufs=8, out_bufs=3,
)


EMIT_LOG = []  # (label, first_inst_id, last_inst_id) for profiling


def emit_body(nc, tc, pools, xt_ap, wqk_ap, wv_ap, wp_ap, bqk_ap, outp_ap,
              opts=DEFAULT_OPTS):
    (const_pool, qkT_pool, v_pool, yT_pool, es_pool, out_pool,
     rc_pool, bs_pool, sq_psum, y_psum, o_psum, bc_psum) = pools
    do_qkv = "qkv" in opts["stages"]
    do_attn = "attn" in opts["stages"]
    do_cproj = "cproj" in opts["stages"]

    # ---- constant loads on the SP- and Act-HWDGE queues only. xt is loaded
    # in two [128,1024] halves per chunk (2KB bf16 partition lines -- at the
    # DMA efficiency threshold) ordered so the mc=0/1 inputs land first. ----
    bqk = const_pool.tile([128, 4], F32, tag="bqk")
    nc.sync.dma_start(bqk[:], bqk_ap[:])
    xt, wqk, wv = [], [], []
    for cc in range(CC):
        xtc = const_pool.tile([128, T], BF, tag=f"xt{cc}")
        xt.append(xtc)
        wqc = const_pool.tile([128, 512], BF, tag=f"wqk{cc}")
        (nc.sync if cc % 2 == 1 else nc.scalar).dma_start(
            wqc[:], wqk_ap[cc * 128:(cc + 1) * 128, :])
        wqk.append(wqc)
        (nc.sync if cc % 2 == 0 else nc.scalar).dma_start(
            xt[cc][:, 0:1024], xt_ap[cc * 128:(cc + 1) * 128, 0:1024])
    for cc in range(CC):
        wvc = const_pool.tile([128, 256], BF, tag=f"wv{cc}")
        (nc.sync if cc % 2 == 1 else nc.scalar).dma_start(
            wvc[:], wv_ap[cc * 128:(cc + 1) * 128, :])
        wv.append(wvc)
    for cc in range(CC):
        (nc.sync if cc % 2 == 0 else nc.scalar).dma_start(
            xt[cc][:, 1024:2048], xt_ap[cc * 128:(cc + 1) * 128, 1024:2048])
    wp = const_pool.tile([128, 2 * 1024], BF, tag="wp")
    for p2 in range(2):
        nc.sync.dma_start(wp[:, p2 * 1024:(p2 + 1) * 1024],
                          wp_ap[p2 * 128:(p2 + 1) * 128, :])

    qkT = qkT_pool.tile([128, 4 * T], BF, tag="qkT")   # q pair0, q pair1, k pair0, k pair1
    v_all = v_pool.tile([128, MB * 260], BF, tag="v")  # per m-block: 4x(64 v cols + ones col)
    yT = yT_pool.tile([128, 2 * T], BF, tag="yT")      # per pair: [hd, m]
    ones = const_pool.tile([1, 64], BF, tag="ones")
    nc.gpsimd.memset(ones[:], 1.0)
    # lower-triangular (keep j<=m) bf16 mask for diagonal score blocks
    cmask = const_pool.tile([128, 128], BF, tag="cmask")
    nc.gpsimd.memset(cmask[:], 1.0)
    nc.gpsimd.affine_select(
        out=cmask[:], in_=cmask[:], compare_op=mybir.AluOpType.is_ge,
        fill=0.0, base=0, pattern=[[1, 128]], channel_multiplier=-1)

    # ones columns of v_all (one memset; disjoint from the v columns)
    nc.gpsimd.memset(
        v_all.rearrange("p (mb h x) -> p mb h x", h=4, x=65)[:, :, :, 64:65], 1.0)

    def _qkv_pg_half(ps, mc, pg, half):
        pp = 2 * pg + half
        for cc in range(CC):
            nc.tensor.matmul(
                ps[:, half * 512:(half + 1) * 512],
                lhsT=wqk[cc][:, pp * 128:(pp + 1) * 128],
                rhs=xt[cc][:, mc * 512:(mc + 1) * 512],
                start=(cc == 0), stop=(cc == CC - 1))

    def qkv_pg_pieces(mc, pg):
        # pair-group pg: pp = 2*pg, 2*pg+1 (q pairs for pg=0, k pairs for pg=1)
        st = {}

        def piece0():
            ps_t = sq_psum.tile([128, 1024], F32, tag="sq")
            st["ps"] = ps_t
            _qkv_pg_half(ps_t, mc, pg, 0)

        def piece1():
            ps = st["ps"]
            _qkv_pg_half(ps, mc, pg, 1)
            for half in range(2):
                pp = 2 * pg + half
                nc.vector.tensor_scalar_add(
                    qkT[:, pp * T + mc * 512: pp * T + (mc + 1) * 512],
                    ps[:, half * 512:(half + 1) * 512], bqk[:, pp:pp + 1])

        return [(f"qkv{mc}:pg{pg}a", piece0), (f"qkv{mc}:pg{pg}b", piece1)]

    def _qkv_v_half(ps, mc, qlo):
        for q in (qlo, qlo + 1):
            mb = 4 * mc + q
            for cc in range(CC):
                nc.tensor.matmul(
                    ps[:, q * 256:(q + 1) * 256],
                    lhsT=xt[cc][:, mb * 128:(mb + 1) * 128],
                    rhs=wv[cc][:],
                    start=(cc == 0), stop=(cc == CC - 1))

    def qkv_v_pieces(mc):
        st = {}

        def piece0():
            ps_t = sq_psum.tile([128, 1024], F32, tag="sq")
            st["ps"] = ps_t
            _qkv_v_half(ps_t, mc, 0)
            nc.vector.tensor_copy(
                v_all.rearrange("p (mb h x) -> p mb h x", h=4, x=65)[
                    :, 4 * mc:4 * mc + 2, :, 0:64],
                ps_t[:, 0:512].rearrange("p (q h x) -> p q h x", h=4, x=64))

        def piece1():
            ps = st["ps"]
            _qkv_v_half(ps, mc, 2)
            nc.vector.tensor_copy(
                v_all.rearrange("p (mb h x) -> p mb h x", h=4, x=65)[
                    :, 4 * mc + 2:4 * mc + 4, :, 0:64],
                ps[:, 512:1024].rearrange("p (q h x) -> p q h x", h=4, x=64))

        return [(f"qkv{mc}:va", piece0), (f"qkv{mc}:vb", piece1)]

    def emit_qkv_all(mc):
        for lb, fn in qkv_pg_pieces(mc, 0) + qkv_pg_pieces(mc, 1) \
                + qkv_v_pieces(mc):
            fn()

    def _cproj_piece(mb, nch):
        op = o_psum.tile([128, 512], F32, tag="opsum")
        for pp2 in range(2):
            nc.tensor.matmul(
                op[:],
                lhsT=yT[:, pp2 * T + mb * 128: pp2 * T + (mb + 1) * 128],
                rhs=wp[:, pp2 * 1024 + nch * 512: pp2 * 1024 + (nch + 1) * 512],
                start=(pp2 == 0), stop=(pp2 == 1))
        ob = out_pool.tile([128, 512], BF, tag="outsb")
        nc.vector.tensor_copy(ob[:], op[:])
        nc.sync.dma_start(
            outp_ap[mb * 128:(mb + 1) * 128, nch * 512:(nch + 1) * 512],
            ob[:])

    def cproj_pieces(mb):
        return [(f"cproj:mb{mb}n{nch}",
                 lambda nch=nch: _cproj_piece(mb, nch)) for nch in range(2)]

    def emit_attn_pair(mc, h, p):
        pp, half = h // 2, h % 2
        prow = 64 * half
        qoff = pp * T
        koff = (2 + pp) * T
        yp = yps[h]
        n_pairs = 2 * mc + 2
        jb0 = 2 * p
        if p < 2 * mc:
            off0 = off1 = 0
            w0 = w1 = 512
        else:
            off0 = (jb0 - 4 * mc) * 128
            w0 = 512 - off0
            w1 = w0 - 128
            off1 = off0 + 128
        sp = sq_psum.tile([128, 1024], F32, tag="sq")
        nc.tensor.matmul(
            sp[:, 0:w0],
            lhsT=qkT[prow:prow + 64,
                     koff + jb0 * 128: koff + (jb0 + 1) * 128],
            rhs=qkT[prow:prow + 64,
                    qoff + mc * 512 + off0: qoff + mc * 512 + off0 + w0],
            start=True, stop=True)
        nc.tensor.matmul(
            sp[:, w0:w0 + w1],
            lhsT=qkT[prow:prow + 64,
                     koff + (jb0 + 1) * 128: koff + (jb0 + 2) * 128],
            rhs=qkT[prow:prow + 64,
                    qoff + mc * 512 + off1: qoff + mc * 512 + off1 + w1],
            start=True, stop=True)
        es = es_pool.tile([128, 1024], BF, tag="es")
        nc.scalar.activation(es[:, 0:w0 + w1], sp[:, 0:w0 + w1],
                             mybir.ActivationFunctionType.Exp)
        if p >= 2 * mc:
            # zero the upper-triangular parts of both 128-col diagonal
            # triangles (jb0 at col 0, jb1 at col w0)
            nc.vector.tensor_mul(es[:, 0:128], es[:, 0:128], cmask[:])
            nc.vector.tensor_mul(es[:, w0:w0 + 128], es[:, w0:w0 + 128],
                                 cmask[:])
        nc.tensor.matmul(
            yp[:, off0:512],
            lhsT=v_all[:, jb0 * 260 + h * 65: jb0 * 260 + (h + 1) * 65],
            rhs=es[:, 0:w0],
            start=(p == 0), stop=False)
        nc.tensor.matmul(
            yp[:, off1:512],
            lhsT=v_all[:, (jb0 + 1) * 260 + h * 65:
                       (jb0 + 1) * 260 + (h + 1) * 65],
            rhs=es[:, w0:w0 + w1],
            start=False, stop=(p == n_pairs - 1))

    def emit_normalize(mc, h):
        pp, half = h // 2, h % 2
        prow = 64 * half
        yp = yps[h]
        rc = rc_pool.tile([1, 512], BF, tag="recip")
        with nc.allow_low_precision(reason="1/l broadcastee; bf16 ok"):
            nc.vector.reciprocal(rc[:], yp[64:65, :])
        bc = bc_psum.tile([64, 512], F32, tag="bcps")
        nc.tensor.matmul(bc[:], lhsT=ones[:], rhs=rc[:], start=True, stop=True)
        bs = bs_pool.tile([64, 512], F32, tag="bs")
        nc.scalar.activation(bs[:], bc[:], mybir.ActivationFunctionType.Copy)
        nc.vector.tensor_mul(
            yT[prow:prow + 64, pp * T + mc * 512: pp * T + (mc + 1) * 512],
            yp[0:64, :], bs[:])

    if not do_attn:
        if do_qkv:
            for mc in range(MC):
                emit_qkv_all(mc)
        return

    def logged(label, fn, *args):
        i0 = nc.next_id()
        fn(*args)
        EMIT_LOG.append((label, i0, nc.next_id()))

    # qkv(0) up front; then per mc: attention pairs with qkv(mc+1) and
    # cproj(mc-1) interleaved as PE filler between pairs.
    logged("qkv0", emit_qkv_all, 0)
    yps = {}
    for mc in range(MC):
        filler = []
        if do_qkv and mc + 1 < MC:
            filler += qkv_pg_pieces(mc + 1, 0) + qkv_pg_pieces(mc + 1, 1) \
                + qkv_v_pieces(mc + 1)
        if do_cproj and mc > 0:
            for mb in range(4 * (mc - 1), 4 * mc):
                filler += cproj_pieces(mb)
        n_pairs = 2 * mc + 2
        total_pairs = HPC * n_pairs
        stride = max(1, total_pairs // (len(filler) + 1))
        cnt = 0
        for h in range(HPC):
            yp_t = y_psum.tile([65, 512], F32, tag="ypsum")
            yps[h] = yp_t
            for p in range(n_pairs):
                logged(f"attn{mc}:h{h}p{p}", emit_attn_pair, mc, h, p)
                cnt += 1
                if cnt % stride == 0 and filler:
                    lb, fn = filler.pop(0)
                    logged(lb, fn)
            logged(f"norm{mc}:h{h}", emit_normalize, mc, h)
        while filler:
            lb, fn = filler.pop(0)
            logged(lb, fn)
    if do_cproj:
        for mb in range(4 * (MC - 1), 4 * MC):
            for lb, fn in cproj_pieces(mb):
                logged(lb, fn)


def _build_pools(nc, tc, ctx, opts):
    const_pool = ctx.enter_context(tc.tile_pool(name="const", bufs=1))
    qkT_pool = ctx.enter_context(tc.tile_pool(name="qkT", bufs=1))
    v_pool = ctx.enter_context(tc.tile_pool(name="v", bufs=1))
    yT_pool = ctx.enter_context(tc.tile_pool(name="yT", bufs=1))
    es_pool = ctx.enter_context(tc.tile_pool(name="es", bufs=opts["es_bufs"]))
    out_pool = ctx.enter_context(tc.tile_pool(name="outsb", bufs=opts["out_bufs"]))
    rc_pool = ctx.enter_context(tc.tile_pool(name="recip", bufs=2))
    bs_pool = ctx.enter_context(tc.tile_pool(name="bs", bufs=2))
    sq_psum = ctx.enter_context(
        tc.tile_pool(name="sq_ps", bufs=opts["sq_bufs"], space="PSUM"))
    y_psum = ctx.enter_context(
        tc.tile_pool(name="y_ps", bufs=opts["y_bufs"], space="PSUM"))
    o_psum = ctx.enter_context(
        tc.tile_pool(name="o_ps", bufs=opts["o_bufs"], space="PSUM"))
    bc_psum = ctx.enter_context(
        tc.tile_pool(name="bc_ps", bufs=1, space="PSUM"))
    return (const_pool, qkT_pool, v_pool, yT_pool, es_pool, out_pool,
            rc_pool, bs_pool, sq_psum, y_psum, o_psum, bc_psum)


def _declare_io(nc):
    xt_ap = nc.dram_tensor("xt", [C, T], BF, kind="ExternalInput").ap()
    wqk_ap = nc.dram_tensor("wqk", [C, 512], BF, kind="ExternalInput").ap()
    wv_ap = nc.dram_tensor("wv", [C, 256], BF, kind="ExternalInput").ap()
    wp_ap = nc.dram_tensor("wp", [256, 1024], BF, kind="ExternalInput").ap()
    bqk_ap = nc.dram_tensor("bqk", [128, 4], F32, kind="ExternalInput").ap()
    outp_ap = nc.dram_tensor("outp", [T, C], BF, kind="ExternalOutput").ap()
    return xt_ap, wqk_ap, wv_ap, wp_ap, bqk_ap, outp_ap


def build(reps=1, opts=None):
    from contextlib import ExitStack
    opts = {**DEFAULT_OPTS, **(opts or {})}
    nc = bacc.Bacc("TRN2", target_bir_lowering=False, debug=False)
    aps = _declare_io(nc)
    with tile.TileContext(nc) as tc:
        with ExitStack() as ctx:
            pools = _build_pools(nc, tc, ctx, opts)
            for _ in range(reps):
                emit_body(nc, tc, pools, *aps, opts)
    nc.compile()
    return nc


def build_looped(n_iters, opts=None):
    """Body wrapped in a hardware For_i loop, for wall-clock slope timing."""
    from contextlib import ExitStack
    opts = {**DEFAULT_OPTS, **(opts or {})}
    nc = bacc.Bacc("TRN2", target_bir_lowering=False, debug=False)
    aps = _declare_io(nc)
    with tile.TileContext(nc) as tc:
        with ExitStack() as ctx:
            pools = _build_pools(nc, tc, ctx, opts)
            with tc.For_i(0, n_iters, 1):
                emit_body(nc, tc, pools, *aps, opts)
    nc.compile()
    return nc


_NC_CACHE = {}


def _get_nc(reps=1, opts=None):
    key = (reps, tuple(sorted((opts or {}).items())))
    if key not in _NC_CACHE:
        _NC_CACHE[key] = build(reps, opts)
    return _NC_CACHE[key]


def make_in_maps(x, w_attn, b_attn, w_proj):
    x = np.asarray(x, np.float32)
    w_attn = np.asarray(w_attn, np.float32)
    b_attn = np.asarray(b_attn, np.float32)
    in_maps = []
    xt_b = [np.ascontiguousarray(x[b].T).astype(BF16) for b in range(B)]
    for c in range(N_CORES):
        b, g = divmod(c, GROUPS)
        h0 = HPC * g
        qs, ks = h0 * D, C + h0 * D
        wqk = np.concatenate([
            0.125 * w_attn[:, qs:qs + 128], 0.125 * w_attn[:, qs + 128:qs + 256],
            w_attn[:, ks:ks + 128], w_attn[:, ks + 128:ks + 256]], axis=1).astype(BF16)
        wv = w_attn[:, 2 * C + g * 256: 2 * C + (g + 1) * 256].astype(BF16)
        wp = np.asarray(w_proj, np.float32)[g * 256:(g + 1) * 256, :].astype(BF16)
        bqk = np.stack([
            0.125 * b_attn[qs:qs + 128], 0.125 * b_attn[qs + 128:qs + 256],
            b_attn[ks:ks + 128], b_attn[ks + 128:ks + 256]], axis=1).astype(np.float32)
        in_maps.append({"xt": xt_b[b], "wqk": np.ascontiguousarray(wqk),
                        "wv": np.ascontiguousarray(wv), "wp": np.ascontiguousarray(wp),
                        "bqk": np.ascontiguousarray(bqk)})
    return in_maps


def assemble_output(results, b_attn, w_proj, b_proj):
    b_attn = np.asarray(b_attn, np.float32)
    w_proj = np.asarray(w_proj, np.float32)
    b_proj = np.asarray(b_proj, np.float32)
    extra = b_attn[2 * C:] @ w_proj + b_proj  # v-bias flows through softmax as +bv
    out = np.empty((B, T, C), np.float32)
    for b in range(B):
        acc = results[4 * b]["outp"].astype(np.float32)
        for g in range(1, GROUPS):
            acc = acc + results[4 * b + g]["outp"].astype(np.float32)
        out[b] = acc + extra
    return out


def kernel(x, w_attn, b_attn, w_proj, b_proj):
    nc = _get_nc(reps=1)
    in_maps = make_in_maps(x, w_attn, b_attn, w_proj)
    res = run_bass_kernel_spmd(nc, in_maps, list(range(N_CORES)))
    return assemble_output(res.results, b_attn, w_proj, b_proj)
